# revision 1
# baseline (speedup 1.0000x reference)
"""Trainium2 Bass kernel for nn_EquivariantDecoder.

Data-parallel over 8 NeuronCores (batch sharded, 2048 rows/core).

The execution environment charges a large fixed cost per *unique* NEFF
instruction, while For_i loops re-execute instructions at ~2us per
back-edge.  The kernel is therefore a minimal program (~80
instructions) of dynamic loops:

  - host pre-transposes v into 38 matmul-ready [128, B] slots (bf16)
    and packs all layer weights (block-diagonal per-irrep) into one
    [128, NWC] bf16 tensor;
  - per b-tile of 512 rows: one DMA stages the tile's 38 slots into
    SBUF (the only t-indexed access; inner-loop access patterns are
    m-indexed only, which keeps the symbolic-AP register budget small);
  - each layer's per-m matmuls run under For_i with shared
    (weight-stationary) block-diagonal weights; gates applied in-place
    by DVE so layers 2/3 reuse the same h buffers;
  - layer 4 accumulates all 49 outputs into one PSUM bank; the host
    de-transposes the [49, 2048] per-core result.
"""

import numpy as np
import ml_dtypes
from contextlib import ExitStack

import concourse.bass as bass
import concourse.mybir as mybir
import concourse.tile as tile
from concourse.bass import ds, ts
from concourse import bass_utils

BF16 = mybir.dt.bfloat16
FP32 = mybir.dt.float32
bf = ml_dtypes.bfloat16

# ---------------- problem constants (hardcoded) ----------------
B_FULL = 16384
NCORES = 8
BC = B_FULL // NCORES          # 2048 rows per core
BT = 512                       # b-tile
NT = BC // BT                  # 4

IN_IRREPS = [(256, 0), (128, 1), (128, 2), (64, 3), (64, 4), (64, 5), (64, 6)]
HID_IRREPS = [(64, 0), (64, 1), (64, 2), (32, 3), (32, 4), (32, 5), (32, 6)]
N_SCALARS = 64
N_GATES = 256
D_IN = 3840
D_OUT = 49

IN_OFF = {}
_o = 0
for _mul, _l in IN_IRREPS:
    IN_OFF[_l] = _o
    _o += _mul * (2 * _l + 1)

# v slot map (38 slots of [128, B]): 2 l0-kslots | 5 l1 | 5 l2 | 13 B56 | 13 B34
S_L0 = 0
S_L1 = 2                       # l1 m-slots (m=3,4 zero)
S_L2 = 7
S_B56 = 12                     # rows 0:64 = l6 (all m), rows 64:128 = l5 (m<11)
S_B34 = 25                     # rows 0:64 = l4 (m<9),  rows 64:128 = l3 (m<7)
NSLOT = 38

_BUILD = {}
REPEAT = 1


def _pack_weights(w1, w2, w3, w4):
    """Pack all weights into one [128, NWC] bf16 matrix (columns).
    Returns (wt, col) where col maps name -> column offset."""

    def split_blocks(wflat, in_irr, out_irr):
        mul_in = {l: m for m, l in in_irr}
        blocks = []
        off = 0
        for mo, l in out_irr:
            mi = mul_in[l]
            w = wflat[off:off + mi * mo].reshape(mi, mo) / np.sqrt(mi)
            off += mi * mo
            blocks.append((l, w))
        assert off == wflat.size
        return blocks

    pre_irr = [(N_SCALARS, 0), (N_GATES, 0)] + [(m, l) for m, l in HID_IRREPS if l > 0]
    # gate channel order: gA = [g_l2 | g_l1], gB = [g_l6 | g_l5 | g_l4 | g_l3]
    gperm = ([64 + i for i in range(64)] + [i for i in range(64)] +
             [224 + i for i in range(32)] + [192 + i for i in range(32)] +
             [160 + i for i in range(32)] + [128 + i for i in range(32)])

    segs = []            # (name, [128, w] fp32 array)

    def add(name, arr):
        a = np.zeros((128, arr.shape[1]), np.float32)
        a[:arr.shape[0]] = arr
        segs.append((name, a))

    # ---- layer 1 ----
    b1 = split_blocks(w1, IN_IRREPS, pre_irr)
    ws, wg = b1[0][1], b1[1][1]                      # [256,64], [256,256]
    W10 = np.concatenate([ws, wg[:, gperm]], axis=1)  # [256, 320]
    add("W1_0a", W10[:128])
    add("W1_0b", W10[128:])
    w1l = {l: w for l, w in b1[2:]}
    add("W1_l1", w1l[1])                             # [128, 64] -> l1 out
    add("W1_l2", w1l[2])                             # [128, 64] -> l2 out
    wb56 = np.zeros((128, 64), np.float32)
    wb56[0:64, 0:32] = w1l[6]
    wb56[64:128, 32:64] = w1l[5]
    add("W1_b56", wb56)
    wb34 = np.zeros((128, 64), np.float32)
    wb34[0:64, 0:32] = w1l[4]
    wb34[64:128, 32:64] = w1l[3]
    add("W1_b34", wb34)

    # ---- layers 2, 3 ----
    for name, wflat in (("W2", w2), ("W3", w3)):
        b = split_blocks(wflat, HID_IRREPS, pre_irr)
        ws, wg = b[0][1], b[1][1]                    # [64,64], [64,256]
        add(name + "_0", np.concatenate([ws, wg[:, gperm]], axis=1))   # [64, 320]
        wl = {l: w for l, w in b[2:]}
        wa = np.zeros((128, 128), np.float32)
        wa[0:64, 0:64] = wl[2]
        wa[64:128, 64:128] = wl[1]
        add(name + "_A", wa)
        wb = np.zeros((128, 128), np.float32)
        for j, l in enumerate((6, 5, 4, 3)):
            wb[32 * j:32 * (j + 1), 32 * j:32 * (j + 1)] = wl[l]
        add(name + "_B", wb)

    # ---- layer 4 ----
    b4 = split_blocks(w4, HID_IRREPS, [(1, l) for l in range(7)])
    w4l = {l: w[:, 0] for l, w in b4}
    OUT_OFF = {l: l * l for l in range(7)}
    w40 = np.zeros((64, D_OUT), np.float32)
    w40[:, 0] = w4l[0]
    add("W4_0", w40)
    for m in range(5):
        wa = np.zeros((128, D_OUT), np.float32)
        wa[0:64, OUT_OFF[2] + m] = w4l[2]
        if m < 3:
            wa[64:128, OUT_OFF[1] + m] = w4l[1]
        add(f"W4_A{m}", wa)
    for m in range(13):
        wbm = np.zeros((128, D_OUT), np.float32)
        wbm[0:32, OUT_OFF[6] + m] = w4l[6]
        if m < 11:
            wbm[32:64, OUT_OFF[5] + m] = w4l[5]
        if m < 9:
            wbm[64:96, OUT_OFF[4] + m] = w4l[4]
        if m < 7:
            wbm[96:128, OUT_OFF[3] + m] = w4l[3]
        add(f"W4_B{m}", wbm)

    col = {}
    off = 0
    for name, a in segs:
        col[name] = off
        off += a.shape[1]
    wt = np.concatenate([a for _, a in segs], axis=1).astype(bf)
    return wt, col


def _pack_v(v_raw):
    """[B_FULL, 3840] fp32 -> [128, NSLOT, B_FULL] bf16 slot layout."""
    B = v_raw.shape[0]
    vt = np.zeros((128, NSLOT, B), np.float32)
    v0 = v_raw[:, IN_OFF[0]:IN_OFF[0] + 256]
    vt[:, S_L0 + 0, :] = v0[:, 0:128].T
    vt[:, S_L0 + 1, :] = v0[:, 128:256].T
    for l, base, nm in ((1, S_L1, 3), (2, S_L2, 5)):
        d = 2 * l + 1
        vb = v_raw[:, IN_OFF[l]:IN_OFF[l] + 128 * d].reshape(B, 128, d)
        for m in range(nm):
            vt[:, base + m, :] = vb[:, :, m].T
    for l, base, rows in ((6, S_B56, slice(0, 64)), (5, S_B56, slice(64, 128)),
                          (4, S_B34, slice(0, 64)), (3, S_B34, slice(64, 128))):
        d = 2 * l + 1
        vb = v_raw[:, IN_OFF[l]:IN_OFF[l] + 64 * d].reshape(B, 64, d)
        for m in range(d):
            vt[rows, base + m, :] = vb[:, :, m].T
    return vt.astype(bf)


def _split_excess_waits(nc, max_waits=1):
    """Walrus accepts only one sem-wait on some ops; hoist excess waits
    onto same-engine NoOps inserted before."""
    for f in nc.m.functions:
        for bb in f.blocks:
            newlist = []
            changed = False
            for ins in bb.instructions:
                si = ins.sync_info
                waits = list(si.on_wait) if (si and si.on_wait) else []
                if len(waits) > max_waits:
                    extras, keep = waits[:-max_waits], waits[-max_waits:]
                    for k in range(0, len(extras), max_waits):
                        nop = mybir.InstNoOp(
                            name=f"{ins.name}_waitnop{k}", ins=[], outs=[],
                            engine=ins.engine)
                        nop.sync_info = mybir.SyncInfo(
                            on_wait=extras[k:k + max_waits], on_update=[])
                        nc.register_instruction(nop)
                        newlist.append(nop)
                    ins.sync_info = mybir.SyncInfo(
                        on_wait=keep,
                        on_update=list(si.on_update) if si.on_update else [])
                    changed = True
                newlist.append(ins)
            if changed:
                bb.instructions[:] = newlist
    return nc


def _build_program(col, repeat=1):
    nc = bass.Bass("TRN2", target_bir_lowering=False, debug=False)
    NWC = max(col.values()) + D_OUT   # W4_B12 is last
    vt_d = nc.dram_tensor("vt", [128, NSLOT, BC], BF16, kind="ExternalInput").ap()
    wt_d = nc.dram_tensor("wt", [128, NWC], BF16, kind="ExternalInput").ap()
    out_d = nc.dram_tensor("out", [D_OUT, BC], FP32, kind="ExternalOutput").ap()

    Sig = mybir.ActivationFunctionType.Sigmoid
    Silu = mybir.ActivationFunctionType.Silu
    Mult = mybir.AluOpType.mult

    with tile.TileContext(nc) as tc:
        with ExitStack() as ctx:
            pool = ctx.enter_context(tc.tile_pool(name="p", bufs=1))
            pp = ctx.enter_context(tc.tile_pool(name="ps", bufs=1, space="PSUM"))

            wt = pool.tile([128, NWC], BF16, tag="wt")
            nc.sync.dma_start(out=wt, in_=wt_d)

            vs = pool.tile([128, NSLOT * BT], BF16, tag="vs")   # staged b-tile
            out49 = pool.tile([D_OUT, BC], FP32, tag="out49")
            h0a = pool.tile([64, BT], BF16, tag="h0a")
            h0b = pool.tile([64, BT], BF16, tag="h0b")
            h0c = pool.tile([64, BT], BF16, tag="h0c")
            hA = pool.tile([128, 5 * BT], BF16, tag="hA")
            hB = pool.tile([128, 13 * BT], BF16, tag="hB")
            gAB1 = pool.tile([128, 2, BT], BF16, tag="gAB1")
            gAB2 = pool.tile([128, 2, BT], BF16, tag="gAB2")
            gAB3 = pool.tile([128, 2, BT], BF16, tag="gAB3")

            z0 = pp.tile([128, 3, BT], FP32, tag="z0")
            zP1 = pp.tile([128, BT], FP32, tag="zP1")
            zP2 = pp.tile([128, BT], FP32, tag="zP2")
            zP3 = pp.tile([128, BT], FP32, tag="zP3")
            z4 = pp.tile([D_OUT, BT], FP32, tag="z4")

            def W(name, w):
                c = col[name]
                return wt[:, c:c + w]

            def W64(name, w):
                c = col[name]
                return wt[0:64, c:c + w]

            mm = nc.tensor.matmul

            # static slot views of the staged tile
            def vslot(s):
                return vs[:, s * BT:(s + 1) * BT]

            vsA1 = vs[:, S_L1 * BT:(S_L1 + 5) * BT]
            vsA2 = vs[:, S_L2 * BT:(S_L2 + 5) * BT]
            vsB56 = vs[:, S_B56 * BT:(S_B56 + 13) * BT]
            vsB34 = vs[:, S_B34 * BT:(S_B34 + 13) * BT]

            def emit_body(t):
                nc.sync.dma_start(out=vs, in_=vt_d[:, :, ds(t * BT, BT)])

                # ---- gate chain: z0 of layers 1..3 (independent of A/B parts) ----
                for k in range(2):
                    wk = ("W1_0a", "W1_0b")[k]
                    st, sp = (k == 0), (k == 1)
                    x = vslot(S_L0 + k)
                    mm(z0[0:64, 0, :], W(wk, 320)[:, 0:64], x, start=st, stop=sp)
                    mm(z0[:, 1, :], W(wk, 320)[:, 64:192], x, start=st, stop=sp)
                    mm(z0[:, 2, :], W(wk, 320)[:, 192:320], x, start=st, stop=sp)
                nc.scalar.activation(gAB1, z0[:, 1:3, :], Sig)
                nc.scalar.activation(h0a, z0[0:64, 0, :], Silu)
                for Wn, gg, hin, hout in (("W2", gAB2, h0a, h0b),
                                          ("W3", gAB3, h0b, h0c)):
                    w0 = W64(Wn + "_0", 320)
                    mm(z0[0:64, 0, :], w0[:, 0:64], hin, start=True, stop=True)
                    mm(z0[:, 1, :], w0[:, 64:192], hin, start=True, stop=True)
                    mm(z0[:, 2, :], w0[:, 192:320], hin, start=True, stop=True)
                    nc.scalar.activation(gg, z0[:, 1:3, :], Sig)
                    nc.scalar.activation(hout, z0[0:64, 0, :], Silu)

                # ---- A superloop: slot m through layers 1->2->3 ----
                with tc.For_i(0, 5, 1) as m:
                    mm(zP1[0:64, :], W("W1_l2", 64), vsA2[:, ds(m * BT, BT)],
                       start=True, stop=True, tile_position=(0, 0))
                    mm(zP1[64:128, :], W("W1_l1", 64), vsA1[:, ds(m * BT, BT)],
                       start=True, stop=True, tile_position=(0, 64))
                    nc.vector.tensor_tensor(out=hA[:, ds(m * BT, BT)], in0=zP1,
                                            in1=gAB1[:, 0, :], op=Mult)
                    mm(zP2, W("W2_A", 128), hA[:, ds(m * BT, BT)],
                       start=True, stop=True)
                    nc.vector.tensor_tensor(out=hA[:, ds(m * BT, BT)], in0=zP2,
                                            in1=gAB2[:, 0, :], op=Mult)
                    mm(zP3, W("W3_A", 128), hA[:, ds(m * BT, BT)],
                       start=True, stop=True)
                    nc.vector.tensor_tensor(out=hA[:, ds(m * BT, BT)], in0=zP3,
                                            in1=gAB3[:, 0, :], op=Mult)

                # ---- B superloop ----
                with tc.For_i(0, 13, 1) as m:
                    mm(zP1[0:64, :], W("W1_b56", 64), vsB56[:, ds(m * BT, BT)],
                       start=True, stop=True, tile_position=(0, 0))
                    mm(zP1[64:128, :], W("W1_b34", 64), vsB34[:, ds(m * BT, BT)],
                       start=True, stop=True, tile_position=(0, 64))
                    nc.vector.tensor_tensor(out=hB[:, ds(m * BT, BT)], in0=zP1,
                                            in1=gAB1[:, 1, :], op=Mult)
                    mm(zP2, W("W2_B", 128), hB[:, ds(m * BT, BT)],
                       start=True, stop=True)
                    nc.vector.tensor_tensor(out=hB[:, ds(m * BT, BT)], in0=zP2,
                                            in1=gAB2[:, 1, :], op=Mult)
                    mm(zP3, W("W3_B", 128), hB[:, ds(m * BT, BT)],
                       start=True, stop=True)
                    nc.vector.tensor_tensor(out=hB[:, ds(m * BT, BT)], in0=zP3,
                                            in1=gAB3[:, 1, :], op=Mult)

                # ---------------- layer 4 ----------------
                mm(z4, W64("W4_0", D_OUT), h0c, start=True, stop=False)
                for m in range(5):
                    mm(z4, W(f"W4_A{m}", D_OUT), hA[:, m * BT:(m + 1) * BT],
                       start=False, stop=False)
                for m in range(13):
                    mm(z4, W(f"W4_B{m}", D_OUT), hB[:, m * BT:(m + 1) * BT],
                       start=False, stop=(m == 12))
                nc.vector.tensor_copy(out49[:, ts(t, BT)], z4)

            if repeat == 1:
                with tc.For_i(0, NT, 1) as t:
                    emit_body(t)
            else:
                with tc.For_i(0, repeat, 1) as r:
                    with tc.For_i(0, NT, 1) as t:
                        emit_body(t)

            nc.sync.dma_start(out=out_d, in_=out49)

    _split_excess_waits(nc)
    return nc


def _get_nc(col):
    key = ("nc", REPEAT)
    if key not in _BUILD:
        _BUILD[key] = _build_program(col, repeat=REPEAT)
    return _BUILD[key]


def kernel(v_raw, w1, w2, w3, w4):
    wt, col = _pack_weights(np.asarray(w1, np.float32), np.asarray(w2, np.float32),
                            np.asarray(w3, np.float32), np.asarray(w4, np.float32))
    nc = _get_nc(col)
    vt = _pack_v(np.asarray(v_raw, np.float32))     # [128, NSLOT, B_FULL] bf16
    in_maps = []
    for c in range(NCORES):
        vc = np.ascontiguousarray(vt[:, :, c * BC:(c + 1) * BC])
        in_maps.append({"vt": vc, "wt": wt})
    res = bass_utils.run_bass_kernel_spmd(nc, in_maps, core_ids=list(range(NCORES)))
    global LAST_RESULT
    LAST_RESULT = res
    full = np.empty((B_FULL, D_OUT), np.float32)
    for c in range(NCORES):
        full[c * BC:(c + 1) * BC, :] = res.results[c]["out"].T
    return full.reshape(B_FULL, D_OUT, 1)



# revision 23
# speedup vs baseline: 4.3765x; 4.3765x over previous
"""Trainium2 Bass kernel for nn_EquivariantDecoder.

Data-parallel over 8 NeuronCores (batch sharded, 2048 rows/core).

Fully unrolled program (no For_i loops: each back-edge costs an
all-engine barrier ~2us).  Per 512-row tile:

  - host pre-transposes v into 30 tight matmul-ready [128, BT] slots
    (bf16) per tile; one contiguous DMA per tile (triple-buffered);
  - layer outputs live in 8 PSUM banks: 3 z0 chunks (T1=[g2;g1],
    T2=[g5;g6;g4;g3], T3=[scalars]), 4 rotating hidden banks, 1 L4;
  - gates: Act engine sigmoids z0 psum -> bf16 SBUF tiles G1/G2/G3;
    silu(s) computed as s * sigmoid(s) (keeps Act on one act table);
  - hidden banks [128, BT] are drained psum->SBUF with the gate
    multiply fused (TensorTensor), spread across DVE (1x from psum),
    Pool (0.42 eff), and Act-copy + DVE 2x-bf16 assists;
  - per-irrep weights are packed block-diagonally so every PE pass is
    [<=128K, <=128P] x BT; weight loads are free; 83 passes/tile.
"""

import numpy as np
import ml_dtypes
from contextlib import ExitStack

import concourse.bass as bass
import concourse.mybir as mybir
import concourse.tile as tile
from concourse.bass import ds, ts
from concourse import bass_utils

BF16 = mybir.dt.bfloat16
FP32 = mybir.dt.float32
bf = ml_dtypes.bfloat16

# ---------------- problem constants (hardcoded) ----------------
B_FULL = 16384
NCORES = 8
BC = B_FULL // NCORES          # 2048 rows per core
BT = 512                       # b-tile
NT = BC // BT                  # 4

IN_IRREPS = [(256, 0), (128, 1), (128, 2), (64, 3), (64, 4), (64, 5), (64, 6)]
HID_IRREPS = [(64, 0), (64, 1), (64, 2), (32, 3), (32, 4), (32, 5), (32, 6)]
D_OUT = 49
NSLOT = 30

IN_OFF = {}
_o = 0
for _mul, _l in IN_IRREPS:
    IN_OFF[_l] = _o
    _o += _mul * (2 * _l + 1)

HID_MUL = {l: m for m, l in HID_IRREPS}
IN_MUL = {l: m for m, l in IN_IRREPS}

# v slot map (30 slots of [128, BT]):
#  0,1: l0 channels 0:128 / 128:256
#  2..6: l2 m=0..4 (128 ch)
#  7..9: l1 m=0..2 (128 ch)
#  10..20: [l6_m(64); l5_m(64)] m=0..10
#  21: [l6_11; l4_7]   22: [l6_12; l4_8]
#  23..29: [l4_m(64); l3_m(64)] m=0..6
_BP = []
for _m in range(7):
    _BP += [("p", (6, _m), (5, _m)), ("p", (4, _m), (3, _m))]
_BP += [("p", (6, 7), (5, 7)), ("p", (6, 8), (5, 8)),
        ("p", (6, 9), (5, 9)), ("p", (6, 10), (5, 10)),
        ("p", (6, 11), (4, 7)), ("p", (6, 12), (4, 8))]
V_SLOTS = ([("l0k", 0, None), ("l0k", 1, None)]
           + [("m", 2, m) for m in range(5)]      # l2
           + [("m", 1, m) for m in range(3)]      # l1
           + _BP)

# hidden banks (14): row layout = 2 or 4 (l, m, size) groups
BANK_GROUPS = (
    [[(2, m, 64), (1, m, 64)] for m in range(3)]
    + [[(2, 3, 64), (2, 4, 64)]]
    + [[(5, m, 32), (6, m, 32), (4, m, 32), (3, m, 32)] for m in range(7)]
    + [[(5, 7, 32), (6, 7, 32), (5, 8, 32), (6, 8, 32)],
       [(5, 9, 32), (6, 9, 32), (5, 10, 32), (6, 10, 32)],
       [(6, 11, 32), (4, 7, 32), (6, 12, 32), (4, 8, 32)]]
)
NBANK = 14

# layer-1 passes per bank: (weight name, v slot) for upper/lower half
L1_PASSES = (
    [[("W1_l2", 2 + m), ("W1_l1", 7 + m)] for m in range(3)]
    + [[("W1_l2", 5), ("W1_l2", 6)]]
    + [[("W56", 10 + 2 * m), ("W43", 11 + 2 * m)] for m in range(7)]
    + [[("W56", 24), ("W56", 25)],
       [("W56", 26), ("W56", 27)],
       [("W64", 28), ("W64", 29)]]
)

# layer-2/3 block-diag weight name per bank
L23_MAT = (["A21"] * 3 + ["A22"] + ["B5643"] * 7 + ["B5656"] * 2 + ["B6464"])

# gate windows: ("G1"|"G2", lo, hi, split?)  G1=[g2;g1] G2=[g5;g6;g4;g3]
BANK_GATE = ([("G1", 0, 128, False)] * 3 + [("G1", 0, 64, True)]
             + [("G2", 0, 128, False)] * 7 + [("G2", 0, 64, True)] * 2
             + [("G2", 32, 96, True)])

# drain jobs per bank: list of (kind, half, gate, lo, hi)
#  kind: 'd' DVE TT, 'p' Pool TT, 'a' Act copy + DVE 2x TT,
#        'a2' Act copy + two half 2x TTs, 'dh'/'ph' half TT
# kinds: 'd' DVE TT from psum; 'a' Act copy + DVE 2x TT;
#        'ap' Act copy + Pool TT (GPSIMD cannot read PSUM)
BANK_JOBS = (
    [[("ap", None, "G1", 0, 128)], [("ap", None, "G1", 0, 128)],
     [("d", None, "G1", 0, 128)], [("a", None, "G1x", 0, 128)]]
    + [[("d", None, "G2", 0, 128)], [("d", None, "G2", 0, 128)]]   # 4,5
    + [[("ap", None, "G2", 0, 128)], [("d", None, "G2", 0, 128)]]  # 6,7
    + [[("d", None, "G2", 0, 128)], [("d", None, "G2", 0, 128)],
       [("d", None, "G2", 0, 128)]]                                # 8,9,10
    + [[("ap", None, "G2x", 0, 128)],                              # bank 11
       [("ap", None, "G2x", 0, 128)],                              # bank 12
       [("a", None, "G2y", 0, 128)]]                               # bank 13
)


# gate column windows in the reference's 256-wide gate block
GCOL = {1: (0, 64), 2: (64, 128), 3: (128, 160), 4: (160, 192),
        5: (192, 224), 6: (224, 256)}

_BUILD = {}


def _split_blocks(wflat, in_irr, out_irr):
    mul_in = {l: m for m, l in in_irr}
    blocks = []
    off = 0
    for mo, l in out_irr:
        mi = mul_in[l]
        w = wflat[off:off + mi * mo].reshape(mi, mo) / np.sqrt(mi)
        off += mi * mo
        blocks.append((l, w))
    assert off == wflat.size
    return blocks


def _pack_weights(w1, w2, w3, w4):
    """Pack all weights into one [128, NW] bf16 matrix.
    Returns (wt, col: name -> (col offset, width))."""
    pre_irr = [(64, 0), (256, 0)] + [(m, l) for m, l in HID_IRREPS if l > 0]

    segs = []

    def add(name, arr):
        a = np.zeros((128, arr.shape[1]), np.float32)
        a[:arr.shape[0]] = arr
        segs.append((name, a))

    def z0_chunks(pfx, wflat, in_irr):
        b = _split_blocks(wflat, in_irr, pre_irr)
        ws, wg = b[0][1], b[1][1]
        t1 = np.concatenate([wg[:, GCOL[2][0]:GCOL[2][1]],
                             wg[:, GCOL[1][0]:GCOL[1][1]]], axis=1)
        t2 = np.concatenate([wg[:, GCOL[5][0]:GCOL[5][1]],
                             wg[:, GCOL[6][0]:GCOL[6][1]],
                             wg[:, GCOL[4][0]:GCOL[4][1]],
                             wg[:, GCOL[3][0]:GCOL[3][1]]], axis=1)
        t3 = ws
        K = t1.shape[0]
        if K == 256:
            add(pfx + "T1_k0", t1[0:128]); add(pfx + "T1_k1", t1[128:256])
            add(pfx + "T2_k0", t2[0:128]); add(pfx + "T2_k1", t2[128:256])
            add(pfx + "T3_k0", t3[0:128]); add(pfx + "T3_k1", t3[128:256])
        else:
            add(pfx + "T1", t1); add(pfx + "T2", t2); add(pfx + "T3", t3)
        return {l: w for l, w in b[2:]}

    # ---- layer 1 ----
    wl1 = z0_chunks("Z1_", w1, IN_IRREPS)
    add("W1_l2", wl1[2])                      # [128, 64]
    add("W1_l1", wl1[1])
    w56 = np.zeros((128, 64), np.float32)     # K=[l6;l5] -> P=[l5out;l6out]
    w56[64:128, 0:32] = wl1[5]
    w56[0:64, 32:64] = wl1[6]
    add("W56", w56)
    w43 = np.zeros((128, 64), np.float32)     # K=[l4;l3] -> P=[l4out;l3out]
    w43[0:64, 0:32] = wl1[4]
    w43[64:128, 32:64] = wl1[3]
    add("W43", w43)
    w64 = np.zeros((128, 64), np.float32)     # K=[l6;l4] -> P=[l6out;l4out]
    w64[0:64, 0:32] = wl1[6]
    w64[64:128, 32:64] = wl1[4]
    add("W64", w64)

    # ---- layers 2, 3 ----
    for li, wflat in ((2, w2), (3, w3)):
        wl = z0_chunks(f"Z{li}_", wflat, HID_IRREPS)
        mats = {}
        for name in set(L23_MAT):
            mats[name] = np.zeros((128, 128), np.float32)
        for b in range(NBANK):
            mat = mats[L23_MAT[b]]
            r = 0
            for (l, m, sz) in BANK_GROUPS[b]:
                mat[r:r + sz, r:r + sz] = wl[l]
                r += sz
        for name in ("A21", "A22", "B5643", "B5656", "B6464"):
            add(f"L{li}_{name}", mats[name])

    # ---- layer 4 ----
    b4 = _split_blocks(w4, HID_IRREPS, [(1, l) for l in range(7)])
    w4l = {l: w[:, 0] for l, w in b4}
    w40 = np.zeros((64, D_OUT), np.float32)
    w40[:, 0] = w4l[0]
    add("W4_h0", w40)
    for b in range(NBANK):
        m4 = np.zeros((128, D_OUT), np.float32)
        r = 0
        for (l, m, sz) in BANK_GROUPS[b]:
            m4[r:r + sz, l * l + m] = w4l[l]
            r += sz
        add(f"W4_b{b}", m4)

    col = {}
    off = 0
    for name, a in segs:
        col[name] = (off, a.shape[1])
        off += a.shape[1]
    wt = np.concatenate([a for _, a in segs], axis=1).astype(bf)
    return wt, col


def _pack_v(v_raw):
    """[B, 3840] fp32 -> [B // BT, 128, NSLOT * BT] bf16 (tile-major)."""
    B = v_raw.shape[0]
    vt = np.zeros((128, NSLOT, B), np.float32)

    def blk(l):
        mul = IN_MUL[l]
        d = 2 * l + 1
        return v_raw[:, IN_OFF[l]:IN_OFF[l] + mul * d].reshape(B, mul, d)

    blks = {l: blk(l) for l in range(7)}
    for s, spec in enumerate(V_SLOTS):
        kind = spec[0]
        if kind == "l0k":
            k = spec[1]
            vt[:, s, :] = v_raw[:, 128 * k:128 * (k + 1)].T
        elif kind == "m":
            l, m = spec[1], spec[2]
            vt[:, s, :] = blks[l][:, :, m].T
        else:
            (lu, mu), (ll_, ml) = spec[1], spec[2]
            vt[0:64, s, :] = blks[lu][:, :, mu].T
            vt[64:128, s, :] = blks[ll_][:, :, ml].T
    # -> [ntile, 128, NSLOT, BT] contiguous per tile
    nt = B // BT
    vt = vt.reshape(128, NSLOT, nt, BT).transpose(2, 0, 1, 3)
    return np.ascontiguousarray(vt.reshape(nt, 128, NSLOT * BT).astype(bf))


def _split_excess_waits(nc, max_waits=1):
    """Walrus accepts only one sem-wait on some ops; hoist excess waits
    onto same-engine NoOps inserted before."""
    for f in nc.m.functions:
        for bb in f.blocks:
            newlist = []
            changed = False
            for ins in bb.instructions:
                si = ins.sync_info
                waits = list(si.on_wait) if (si and si.on_wait) else []
                if len(waits) > max_waits:
                    extras, keep = waits[:-max_waits], waits[-max_waits:]
                    for k in range(0, len(extras), max_waits):
                        nop = mybir.InstNoOp(
                            name=f"{ins.name}_waitnop{k}", ins=[], outs=[],
                            engine=ins.engine)
                        nop.sync_info = mybir.SyncInfo(
                            on_wait=extras[k:k + max_waits], on_update=[])
                        nc.register_instruction(nop)
                        newlist.append(nop)
                    ins.sync_info = mybir.SyncInfo(
                        on_wait=keep,
                        on_update=list(si.on_update) if si.on_update else [])
                    changed = True
                newlist.append(ins)
            if changed:
                bb.instructions[:] = newlist
    return nc


def _build_program(col):
    nc = bass.Bass("TRN2", target_bir_lowering=False, debug=False)
    NW = max(o + w for o, w in col.values())
    vt_d = nc.dram_tensor("vt", [NT, 128, NSLOT * BT], BF16,
                          kind="ExternalInput").ap()
    wt_d = nc.dram_tensor("wt", [128, NW], BF16, kind="ExternalInput").ap()
    out_d = nc.dram_tensor("out", [D_OUT, BC], FP32, kind="ExternalOutput").ap()

    Sig = mybir.ActivationFunctionType.Sigmoid
    Copy = mybir.ActivationFunctionType.Copy
    Mult = mybir.AluOpType.mult

    with tile.TileContext(nc) as tc:
        with ExitStack() as ctx:
            pool = ctx.enter_context(tc.tile_pool(name="p", bufs=1))
            pp = ctx.enter_context(tc.tile_pool(name="ps", bufs=1, space="PSUM"))

            wt = pool.tile([128, NW], BF16, tag="wt")
            vb = [pool.tile([128, NSLOT * BT], BF16, tag=f"vb{i}", name=f"vb{i}")
                  for i in range(2)]
            # h[li][parity]: gated hidden after layer li+1
            h = [[pool.tile([128, NBANK, BT], BF16, tag=f"h{i}_{q}",
                            name=f"h{i}_{q}") for q in range(2)]
                 for i in range(3)]
            h0 = [[pool.tile([64, BT], BF16, tag=f"h0{i}_{q}",
                             name=f"h0{i}_{q}") for q in range(2)]
                  for i in range(3)]
            G1 = [[pool.tile([128, BT], BF16, tag=f"G1_{i}_{q}",
                             name=f"G1_{i}_{q}") for q in range(2)]
                  for i in range(3)]
            G2 = [[pool.tile([128, BT], BF16, tag=f"G2_{i}_{q}",
                             name=f"G2_{i}_{q}") for q in range(2)]
                  for i in range(3)]
            G3 = [[pool.tile([64, BT], BF16, tag=f"G3_{i}_{q}",
                             name=f"G3_{i}_{q}") for q in range(2)]
                  for i in range(3)]
            G1x = [[pool.tile([128, BT], BF16, tag=f"G1x_{i}_{q}",
                              name=f"G1x_{i}_{q}") for q in range(2)]
                   for i in range(3)]
            G2x = [[pool.tile([128, BT], BF16, tag=f"G2x_{i}_{q}",
                              name=f"G2x_{i}_{q}") for q in range(2)]
                   for i in range(3)]
            G2y = [[pool.tile([128, BT], BF16, tag=f"G2y_{i}_{q}",
                              name=f"G2y_{i}_{q}") for q in range(2)]
                   for i in range(3)]
            SG = [pool.tile([128, BT], BF16, tag=f"SG{i}", name=f"SG{i}")
                  for i in range(4)]
            out49 = pool.tile([D_OUT, BC], FP32, tag="out49")

            PZ = pp.tile([D_OUT, BT], FP32, tag="PZ")
            PB = [pp.tile([128, BT], FP32, tag=f"PB{i}", name=f"PB{i}")
                  for i in range(7)]

            def W(name):
                c, w = col[name]
                return wt[:, c:c + w]

            def W64r(name, rows=64):
                c, w = col[name]
                return wt[0:rows, c:c + w]

            mm = nc.tensor.matmul
            TT = nc.vector.tensor_tensor
            PTT = nc.gpsimd.tensor_tensor

            c1 = col["W64"][0] + col["W64"][1]   # end of layer-1 weights
            nc.sync.dma_start(out=wt[:, 0:c1], in_=wt_d[:, 0:c1])
            nc.sync.dma_start(out=vb[0][:, 0:2 * BT], in_=vt_d[0][:, 0:2 * BT])
            nc.sync.dma_start(out=vb[0][:, 2 * BT:10 * BT],
                              in_=vt_d[0][:, 2 * BT:10 * BT])
            nc.sync.dma_start(out=wt[:, c1:], in_=wt_d[:, c1:])
            for a, b in ((10, 18), (18, 26), (26, 30)):
                nc.sync.dma_start(out=vb[0][:, a * BT:b * BT],
                                  in_=vt_d[0][:, a * BT:b * BT])
            nc.sync.dma_start(out=vb[1][:, 0:10 * BT], in_=vt_d[1][:, 0:10 * BT])
            nc.sync.dma_start(out=vb[1][:, 10 * BT:], in_=vt_d[1][:, 10 * BT:])

            # PE p-state warmup: ramp to max clock while DMAs stream
            dmyw = pool.tile([128, D_OUT], BF16, tag="dmyw")
            dmyx = pool.tile([128, BT], BF16, tag="dmyx")
            nc.vector.memset(dmyw, 0)
            nc.vector.memset(dmyx, 0)
            for i in range(12):
                mm(PZ, dmyw, dmyx, start=(i == 0), stop=(i == 11))

            def vs(t, s):
                return vb[t % 2][:, ds(s * BT, BT)]

            bank_ctr = [0]
            sg_ctr = [0]

            def drain(b, P, hdst, gt):
                for kind, hf, gname, lo, hi in BANK_JOBS[b]:
                    g = gt[gname]
                    if kind in ("a", "ap"):
                        sg = SG[sg_ctr[0] % 4]
                        sg_ctr[0] += 1
                        nc.scalar.activation(sg, P, Copy)
                        e = TT if kind == "a" else PTT
                        e(out=hdst, in0=sg, in1=g[lo:hi, :], op=Mult)
                    else:  # 'd'
                        TT(out=hdst, in0=P, in1=g[lo:hi, :], op=Mult)

            def layer_stream(t, li):
                """Generator of emission thunks for (tile t, layer li in 1..4)."""
                q = t % 2
                if li == 4:
                    def l4(lo, hi, first, last):
                        def go():
                            if first:
                                mm(PZ, W64r("W4_h0"), h0[2][q],
                                   start=True, stop=False)
                            for b in range(lo, hi):
                                mm(PZ, W(f"W4_b{b}"), h[2][q][:, b, :],
                                   start=False, stop=(b == NBANK - 1))
                            if last:
                                nc.vector.tensor_copy(out49[:, ts(t, BT)], PZ)
                        return go
                    yield l4(0, 5, True, False)
                    yield l4(5, 10, False, False)
                    yield l4(10, NBANK, False, True)
                    return

                gli = li - 1
                gt = {"G1": G1[gli][q], "G2": G2[gli][q],
                      "G1x": G1x[gli][q], "G2x": G2x[gli][q],
                      "G2y": G2y[gli][q]}

                def z0():
                    Ps = [PB[(bank_ctr[0] + i) % 7] for i in range(3)]
                    bank_ctr[0] += 3
                    # T3 first: h0 feeds the next layer's z0 chain
                    for P, name, rows in ((Ps[0], "T3", 64),
                                          (Ps[1], "T1", 128),
                                          (Ps[2], "T2", 128)):
                        if li == 1:
                            mm(P[0:rows, :], W(f"Z1_{name}_k0"), vs(t, 0),
                               start=True, stop=False)
                            mm(P[0:rows, :], W(f"Z1_{name}_k1"), vs(t, 1),
                               start=False, stop=True)
                        else:
                            mm(P[0:rows, :], W64r(f"Z{li}_{name}"),
                               h0[gli - 1][q], start=True, stop=True)
                    nc.scalar.activation(G3[gli][q], Ps[0][0:64, :], Sig)
                    nc.scalar.activation(G1[gli][q], Ps[1], Sig)
                    nc.scalar.activation(G2[gli][q], Ps[2], Sig)
                    TT(out=h0[gli][q], in0=Ps[0][0:64, :], in1=G3[gli][q],
                       op=Mult)
                    # duplicated gate windows (DVE 4x copies) for the
                    # repeated-pattern banks
                    g1, g2 = G1[gli][q], G2[gli][q]
                    cp = nc.vector.tensor_copy
                    cp(G1x[gli][q][0:64, :], g1[0:64, :])
                    cp(G1x[gli][q][64:128, :], g1[0:64, :])
                    cp(G2x[gli][q][0:64, :], g2[0:64, :])
                    cp(G2x[gli][q][64:128, :], g2[0:64, :])
                    # [g6;g4] window straddles a 64-partition boundary:
                    # 64-wide accesses must be 64-aligned, so copy in 32s
                    cp(G2y[gli][q][0:32, :], g2[32:64, :])
                    cp(G2y[gli][q][32:64, :], g2[64:96, :])
                    cp(G2y[gli][q][64:96, :], g2[32:64, :])
                    cp(G2y[gli][q][96:128, :], g2[64:96, :])
                yield z0

                def bank(b):
                    def go():
                        P = PB[bank_ctr[0] % 7]
                        bank_ctr[0] += 1
                        if li == 1:
                            (wu, su), (wl, sl) = L1_PASSES[b]
                            mm(P[0:64, :], W(wu), vs(t, su),
                               start=True, stop=True, tile_position=(0, 0))
                            mm(P[64:128, :], W(wl), vs(t, sl),
                               start=True, stop=True, tile_position=(0, 64))
                        else:
                            mm(P, W(f"L{li}_{L23_MAT[b]}"),
                               h[gli - 1][q][:, b, :], start=True, stop=True)
                        drain(b, P, h[gli][q][:, b, :], gt)
                    return go
                for b in range(NBANK):
                    yield bank(b)

            def emit_phase(streams, head_start=7):
                its = [iter(s) for s in streams]
                for _ in range(head_start):
                    for th in its[:1]:
                        nxt = next(th, None)
                        if nxt:
                            nxt()
                alive = True
                while alive:
                    alive = False
                    for it in its:
                        nxt = next(it, None)
                        if nxt:
                            nxt()
                            alive = True

            # wavefront schedule: (tile, layer) pairs per phase
            SCHED = [[(0, 1)], [(0, 2), (1, 1)], [(0, 3), (1, 2)],
                     [(0, 4), (1, 3)], [(1, 4), (2, 1)], [(2, 2), (3, 1)],
                     [(2, 3), (3, 2)], [(2, 4), (3, 3)], [(3, 4)]]
            for pi, phase in enumerate(SCHED):
                if pi == 2:
                    nc.sync.dma_start(out=vb[0], in_=vt_d[2])
                if pi == 3:
                    nc.sync.dma_start(out=vb[1], in_=vt_d[3])
                emit_phase([layer_stream(t, l) for t, l in phase])
            nc.sync.dma_start(out=out_d, in_=out49)

    _split_excess_waits(nc)
    return nc


def _get_nc(col):
    if "nc" not in _BUILD:
        _BUILD["nc"] = _build_program(col)
    return _BUILD["nc"]


def kernel(v_raw, w1, w2, w3, w4):
    wt, col = _pack_weights(np.asarray(w1, np.float32),
                            np.asarray(w2, np.float32),
                            np.asarray(w3, np.float32),
                            np.asarray(w4, np.float32))
    nc = _get_nc(col)
    vt = _pack_v(np.asarray(v_raw, np.float32))   # [B//BT, 128, NSLOT*BT]
    in_maps = []
    for c in range(NCORES):
        vc = np.ascontiguousarray(vt[c * NT:(c + 1) * NT])
        in_maps.append({"vt": vc, "wt": wt})
    res = bass_utils.run_bass_kernel_spmd(nc, in_maps,
                                          core_ids=list(range(NCORES)))
    global LAST_RESULT
    LAST_RESULT = res
    full = np.empty((B_FULL, D_OUT), np.float32)
    for c in range(NCORES):
        full[c * BC:(c + 1) * BC, :] = res.results[c]["out"].T
    return full.reshape(B_FULL, D_OUT, 1)


# revision 30
# speedup vs baseline: 4.4772x; 1.0230x over previous
"""Trainium2 Bass kernel for nn_EquivariantDecoder.

Data-parallel over 8 NeuronCores (batch sharded, 2048 rows/core).

Fully unrolled program (no For_i loops: each back-edge costs an
all-engine barrier ~2us).  Per 512-row tile:

  - host pre-transposes v into 30 tight matmul-ready [128, BT] slots
    (bf16) per tile; one contiguous DMA per tile (triple-buffered);
  - layer outputs live in 8 PSUM banks: 3 z0 chunks (T1=[g2;g1],
    T2=[g5;g6;g4;g3], T3=[scalars]), 4 rotating hidden banks, 1 L4;
  - gates: Act engine sigmoids z0 psum -> bf16 SBUF tiles G1/G2/G3;
    silu(s) computed as s * sigmoid(s) (keeps Act on one act table);
  - hidden banks [128, BT] are drained psum->SBUF with the gate
    multiply fused (TensorTensor), spread across DVE (1x from psum),
    Pool (0.42 eff), and Act-copy + DVE 2x-bf16 assists;
  - per-irrep weights are packed block-diagonally so every PE pass is
    [<=128K, <=128P] x BT; weight loads are free; 83 passes/tile.
"""

import numpy as np
import ml_dtypes
from contextlib import ExitStack

import concourse.bass as bass
import concourse.mybir as mybir
import concourse.tile as tile
from concourse.bass import ds, ts
from concourse import bass_utils

BF16 = mybir.dt.bfloat16
FP32 = mybir.dt.float32
bf = ml_dtypes.bfloat16

# ---------------- problem constants (hardcoded) ----------------
B_FULL = 16384
NCORES = 8
BC = B_FULL // NCORES          # 2048 rows per core
BT = 512                       # b-tile
NT = BC // BT                  # 4

IN_IRREPS = [(256, 0), (128, 1), (128, 2), (64, 3), (64, 4), (64, 5), (64, 6)]
HID_IRREPS = [(64, 0), (64, 1), (64, 2), (32, 3), (32, 4), (32, 5), (32, 6)]
D_OUT = 49
NSLOT = 30

IN_OFF = {}
_o = 0
for _mul, _l in IN_IRREPS:
    IN_OFF[_l] = _o
    _o += _mul * (2 * _l + 1)

HID_MUL = {l: m for m, l in HID_IRREPS}
IN_MUL = {l: m for m, l in IN_IRREPS}

# v slot map (30 slots of [128, BT]):
#  0,1: l0 channels 0:128 / 128:256
#  2..6: l2 m=0..4 (128 ch)
#  7..9: l1 m=0..2 (128 ch)
#  10..20: [l6_m(64); l5_m(64)] m=0..10
#  21: [l6_11; l4_7]   22: [l6_12; l4_8]
#  23..29: [l4_m(64); l3_m(64)] m=0..6
_BP = []
for _m in range(7):
    _BP += [("p", (6, _m), (5, _m)), ("p", (4, _m), (3, _m))]
_BP += [("p", (6, 7), (5, 7)), ("p", (6, 8), (5, 8)),
        ("p", (6, 9), (5, 9)), ("p", (6, 10), (5, 10)),
        ("p", (6, 11), (4, 7)), ("p", (6, 12), (4, 8))]
V_SLOTS = ([("l0k", 0, None), ("l0k", 1, None)]
           + [("m", 2, m) for m in range(5)]      # l2
           + [("m", 1, m) for m in range(3)]      # l1
           + _BP)

# hidden banks (14): row layout = 2 or 4 (l, m, size) groups
BANK_GROUPS = (
    [[(2, m, 64), (1, m, 64)] for m in range(3)]
    + [[(2, 3, 64), (2, 4, 64)]]
    + [[(5, m, 32), (6, m, 32), (4, m, 32), (3, m, 32)] for m in range(7)]
    + [[(5, 7, 32), (6, 7, 32), (5, 8, 32), (6, 8, 32)],
       [(5, 9, 32), (6, 9, 32), (5, 10, 32), (6, 10, 32)],
       [(6, 11, 32), (4, 7, 32), (6, 12, 32), (4, 8, 32)]]
)
NBANK = 14

# layer-1 passes per bank: (weight name, v slot) for upper/lower half
L1_PASSES = (
    [[("W1_l2", 2 + m), ("W1_l1", 7 + m)] for m in range(3)]
    + [[("W1_l2", 5), ("W1_l2", 6)]]
    + [[("W56", 10 + 2 * m), ("W43", 11 + 2 * m)] for m in range(7)]
    + [[("W56", 24), ("W56", 25)],
       [("W56", 26), ("W56", 27)],
       [("W64", 28), ("W64", 29)]]
)

# layer-2/3 block-diag weight name per bank
L23_MAT = (["A21"] * 3 + ["A22"] + ["B5643"] * 7 + ["B5656"] * 2 + ["B6464"])

# gate windows: ("G1"|"G2", lo, hi, split?)  G1=[g2;g1] G2=[g5;g6;g4;g3]
BANK_GATE = ([("G1", 0, 128, False)] * 3 + [("G1", 0, 64, True)]
             + [("G2", 0, 128, False)] * 7 + [("G2", 0, 64, True)] * 2
             + [("G2", 32, 96, True)])

# drain jobs per bank: list of (kind, half, gate, lo, hi)
#  kind: 'd' DVE TT, 'p' Pool TT, 'a' Act copy + DVE 2x TT,
#        'a2' Act copy + two half 2x TTs, 'dh'/'ph' half TT
# kinds: 'd' DVE TT from psum; 'a' Act copy + DVE 2x TT;
#        'ap' Act copy + Pool TT (GPSIMD cannot read PSUM)
BANK_JOBS = (
    [[("ap", None, "G1", 0, 128)], [("ap", None, "G1", 0, 128)],
     [("d", None, "G1", 0, 128)], [("a", None, "G1x", 0, 128)]]
    + [[("d", None, "G2", 0, 128)], [("d", None, "G2", 0, 128)]]   # 4,5
    + [[("ap", None, "G2", 0, 128)], [("d", None, "G2", 0, 128)]]  # 6,7
    + [[("d", None, "G2", 0, 128)], [("d", None, "G2", 0, 128)],
       [("d", None, "G2", 0, 128)]]                                # 8,9,10
    + [[("ap", None, "G2x", 0, 128)],                              # bank 11
       [("ap", None, "G2x", 0, 128)],                              # bank 12
       [("a", None, "G2y", 0, 128)]]                               # bank 13
)


# gate column windows in the reference's 256-wide gate block
GCOL = {1: (0, 64), 2: (64, 128), 3: (128, 160), 4: (160, 192),
        5: (192, 224), 6: (224, 256)}

_BUILD = {}


def _split_blocks(wflat, in_irr, out_irr):
    mul_in = {l: m for m, l in in_irr}
    blocks = []
    off = 0
    for mo, l in out_irr:
        mi = mul_in[l]
        w = wflat[off:off + mi * mo].reshape(mi, mo) / np.sqrt(mi)
        off += mi * mo
        blocks.append((l, w))
    assert off == wflat.size
    return blocks


def _pack_weights(w1, w2, w3, w4):
    """Pack all weights into one [128, NW] bf16 matrix.
    Returns (wt, col: name -> (col offset, width))."""
    pre_irr = [(64, 0), (256, 0)] + [(m, l) for m, l in HID_IRREPS if l > 0]

    segs = []

    def add(name, arr):
        a = np.zeros((128, arr.shape[1]), np.float32)
        a[:arr.shape[0]] = arr
        segs.append((name, a))

    def z0_chunks(pfx, wflat, in_irr):
        b = _split_blocks(wflat, in_irr, pre_irr)
        ws, wg = b[0][1], b[1][1]
        t1 = np.concatenate([wg[:, GCOL[2][0]:GCOL[2][1]],
                             wg[:, GCOL[1][0]:GCOL[1][1]]], axis=1)
        t2 = np.concatenate([wg[:, GCOL[5][0]:GCOL[5][1]],
                             wg[:, GCOL[6][0]:GCOL[6][1]],
                             wg[:, GCOL[4][0]:GCOL[4][1]],
                             wg[:, GCOL[3][0]:GCOL[3][1]]], axis=1)
        t3 = ws
        K = t1.shape[0]
        if K == 256:
            add(pfx + "T1_k0", t1[0:128]); add(pfx + "T1_k1", t1[128:256])
            add(pfx + "T2_k0", t2[0:128]); add(pfx + "T2_k1", t2[128:256])
            add(pfx + "T3_k0", t3[0:128]); add(pfx + "T3_k1", t3[128:256])
        else:
            add(pfx + "T1", t1); add(pfx + "T2", t2); add(pfx + "T3", t3)
        return {l: w for l, w in b[2:]}

    # ---- layer 1 ----
    wl1 = z0_chunks("Z1_", w1, IN_IRREPS)
    add("W1_l2", wl1[2])                      # [128, 64]
    add("W1_l1", wl1[1])
    w56 = np.zeros((128, 64), np.float32)     # K=[l6;l5] -> P=[l5out;l6out]
    w56[64:128, 0:32] = wl1[5]
    w56[0:64, 32:64] = wl1[6]
    add("W56", w56)
    w43 = np.zeros((128, 64), np.float32)     # K=[l4;l3] -> P=[l4out;l3out]
    w43[0:64, 0:32] = wl1[4]
    w43[64:128, 32:64] = wl1[3]
    add("W43", w43)
    w64 = np.zeros((128, 64), np.float32)     # K=[l6;l4] -> P=[l6out;l4out]
    w64[0:64, 0:32] = wl1[6]
    w64[64:128, 32:64] = wl1[4]
    add("W64", w64)

    # ---- layers 2, 3 ----
    for li, wflat in ((2, w2), (3, w3)):
        wl = z0_chunks(f"Z{li}_", wflat, HID_IRREPS)
        mats = {}
        for name in set(L23_MAT):
            mats[name] = np.zeros((128, 128), np.float32)
        for b in range(NBANK):
            mat = mats[L23_MAT[b]]
            r = 0
            for (l, m, sz) in BANK_GROUPS[b]:
                mat[r:r + sz, r:r + sz] = wl[l]
                r += sz
        for name in ("A21", "A22", "B5643", "B5656", "B6464"):
            add(f"L{li}_{name}", mats[name])

    # ---- layer 4 ----
    b4 = _split_blocks(w4, HID_IRREPS, [(1, l) for l in range(7)])
    w4l = {l: w[:, 0] for l, w in b4}
    w40 = np.zeros((64, D_OUT), np.float32)
    w40[:, 0] = w4l[0]
    add("W4_h0", w40)
    for b in range(NBANK):
        m4 = np.zeros((128, D_OUT), np.float32)
        r = 0
        for (l, m, sz) in BANK_GROUPS[b]:
            m4[r:r + sz, l * l + m] = w4l[l]
            r += sz
        add(f"W4_b{b}", m4)

    col = {}
    off = 0
    for name, a in segs:
        col[name] = (off, a.shape[1])
        off += a.shape[1]
    wt = np.concatenate([a for _, a in segs], axis=1).astype(bf)
    return wt, col


def _pack_v(v_raw):
    """[B, 3840] fp32 -> [B // BT, 128, NSLOT * BT] bf16 (tile-major)."""
    B = v_raw.shape[0]
    vt = np.zeros((128, NSLOT, B), np.float32)

    def blk(l):
        mul = IN_MUL[l]
        d = 2 * l + 1
        return v_raw[:, IN_OFF[l]:IN_OFF[l] + mul * d].reshape(B, mul, d)

    blks = {l: blk(l) for l in range(7)}
    for s, spec in enumerate(V_SLOTS):
        kind = spec[0]
        if kind == "l0k":
            k = spec[1]
            vt[:, s, :] = v_raw[:, 128 * k:128 * (k + 1)].T
        elif kind == "m":
            l, m = spec[1], spec[2]
            vt[:, s, :] = blks[l][:, :, m].T
        else:
            (lu, mu), (ll_, ml) = spec[1], spec[2]
            vt[0:64, s, :] = blks[lu][:, :, mu].T
            vt[64:128, s, :] = blks[ll_][:, :, ml].T
    # -> [ntile, 128, NSLOT, BT] contiguous per tile
    nt = B // BT
    vt = vt.reshape(128, NSLOT, nt, BT).transpose(2, 0, 1, 3)
    return np.ascontiguousarray(vt.reshape(nt, 128, NSLOT * BT).astype(bf))


def _split_excess_waits(nc, max_waits=1):
    """Walrus accepts only one sem-wait on some ops; hoist excess waits
    onto same-engine NoOps inserted before."""
    for f in nc.m.functions:
        for bb in f.blocks:
            newlist = []
            changed = False
            for ins in bb.instructions:
                si = ins.sync_info
                waits = list(si.on_wait) if (si and si.on_wait) else []
                if len(waits) > max_waits:
                    extras, keep = waits[:-max_waits], waits[-max_waits:]
                    for k in range(0, len(extras), max_waits):
                        nop = mybir.InstNoOp(
                            name=f"{ins.name}_waitnop{k}", ins=[], outs=[],
                            engine=ins.engine)
                        nop.sync_info = mybir.SyncInfo(
                            on_wait=extras[k:k + max_waits], on_update=[])
                        nc.register_instruction(nop)
                        newlist.append(nop)
                    ins.sync_info = mybir.SyncInfo(
                        on_wait=keep,
                        on_update=list(si.on_update) if si.on_update else [])
                    changed = True
                newlist.append(ins)
            if changed:
                bb.instructions[:] = newlist
    return nc


def _build_program(col):
    nc = bass.Bass("TRN2", target_bir_lowering=False, debug=False)
    NW = max(o + w for o, w in col.values())
    vt_d = nc.dram_tensor("vt", [NT, 128, NSLOT * BT], BF16,
                          kind="ExternalInput").ap()
    wt_d = nc.dram_tensor("wt", [128, NW], BF16, kind="ExternalInput").ap()
    out_d = nc.dram_tensor("out", [D_OUT, BC], FP32, kind="ExternalOutput").ap()

    Sig = mybir.ActivationFunctionType.Sigmoid
    Copy = mybir.ActivationFunctionType.Copy
    Mult = mybir.AluOpType.mult

    with tile.TileContext(nc) as tc:
        with ExitStack() as ctx:
            pool = ctx.enter_context(tc.tile_pool(name="p", bufs=1))
            pp = ctx.enter_context(tc.tile_pool(name="ps", bufs=1, space="PSUM"))

            wt = pool.tile([128, NW], BF16, tag="wt")
            vb = [pool.tile([128, NSLOT * BT], BF16, tag=f"vb{i}", name=f"vb{i}")
                  for i in range(2)]
            # h[li][parity]: gated hidden after layer li+1
            h = [[pool.tile([128, NBANK, BT], BF16, tag=f"h{i}_{q}",
                            name=f"h{i}_{q}") for q in range(2)]
                 for i in range(3)]
            h0 = [[pool.tile([64, BT], BF16, tag=f"h0{i}_{q}",
                             name=f"h0{i}_{q}") for q in range(2)]
                  for i in range(3)]
            G1 = [[pool.tile([128, BT], BF16, tag=f"G1_{i}_{q}",
                             name=f"G1_{i}_{q}") for q in range(2)]
                  for i in range(3)]
            G2 = [[pool.tile([128, BT], BF16, tag=f"G2_{i}_{q}",
                             name=f"G2_{i}_{q}") for q in range(2)]
                  for i in range(3)]
            G3 = [[pool.tile([64, BT], BF16, tag=f"G3_{i}_{q}",
                             name=f"G3_{i}_{q}") for q in range(2)]
                  for i in range(3)]
            G1x = [[pool.tile([128, BT], BF16, tag=f"G1x_{i}_{q}",
                              name=f"G1x_{i}_{q}") for q in range(2)]
                   for i in range(3)]
            G2x = [[pool.tile([128, BT], BF16, tag=f"G2x_{i}_{q}",
                              name=f"G2x_{i}_{q}") for q in range(2)]
                   for i in range(3)]
            G2y = [[pool.tile([128, BT], BF16, tag=f"G2y_{i}_{q}",
                              name=f"G2y_{i}_{q}") for q in range(2)]
                   for i in range(3)]
            SG = [pool.tile([128, BT], BF16, tag=f"SG{i}", name=f"SG{i}")
                  for i in range(4)]
            out49 = pool.tile([D_OUT, BC], FP32, tag="out49")

            PZ = pp.tile([D_OUT, BT], FP32, tag="PZ")
            PB = [pp.tile([128, BT], FP32, tag=f"PB{i}", name=f"PB{i}")
                  for i in range(7)]

            def W(name):
                c, w = col[name]
                return wt[:, c:c + w]

            def W64r(name, rows=64):
                c, w = col[name]
                return wt[0:rows, c:c + w]

            mm = nc.tensor.matmul
            TT = nc.vector.tensor_tensor
            PTT = nc.gpsimd.tensor_tensor

            c1 = col["W64"][0] + col["W64"][1]   # end of layer-1 weights
            nc.sync.dma_start(out=wt[:, 0:c1], in_=wt_d[:, 0:c1])
            nc.sync.dma_start(out=vb[0][:, 0:2 * BT], in_=vt_d[0][:, 0:2 * BT])
            nc.sync.dma_start(out=vb[0][:, 2 * BT:10 * BT],
                              in_=vt_d[0][:, 2 * BT:10 * BT])
            for a, b in ((10, 18), (18, 26), (26, 30)):
                nc.sync.dma_start(out=vb[0][:, a * BT:b * BT],
                                  in_=vt_d[0][:, a * BT:b * BT])
            nc.sync.dma_start(out=wt[:, c1:], in_=wt_d[:, c1:])
            nc.sync.dma_start(out=vb[1][:, 0:10 * BT], in_=vt_d[1][:, 0:10 * BT])
            nc.sync.dma_start(out=vb[1][:, 10 * BT:], in_=vt_d[1][:, 10 * BT:])

            # PE p-state warmup: ramp to max clock while DMAs stream
            dmyw = pool.tile([128, D_OUT], BF16, tag="dmyw")
            dmyx = pool.tile([128, BT], BF16, tag="dmyx")
            nc.vector.memset(dmyw, 0)
            nc.vector.memset(dmyx, 0)
            for i in range(8):
                mm(PZ, dmyw, dmyx, start=(i == 0), stop=(i == 7))

            def vs(t, s):
                return vb[t % 2][:, ds(s * BT, BT)]

            bank_ctr = [0]
            sg_ctr = [0]

            def drain(b, P, hdst, gt):
                for kind, hf, gname, lo, hi in BANK_JOBS[b]:
                    g = gt[gname]
                    if kind in ("a", "ap"):
                        sg = SG[sg_ctr[0] % 4]
                        sg_ctr[0] += 1
                        nc.scalar.activation(sg, P, Copy)
                        e = TT if kind == "a" else PTT
                        e(out=hdst, in0=sg, in1=g[lo:hi, :], op=Mult)
                    else:  # 'd'
                        TT(out=hdst, in0=P, in1=g[lo:hi, :], op=Mult)

            def layer_stream(t, li):
                """Generator of emission thunks for (tile t, layer li in 1..4)."""
                q = t % 2
                if li == 4:
                    def l4(lo, hi, first, last):
                        def go():
                            if first:
                                mm(PZ, W64r("W4_h0"), h0[2][q],
                                   start=True, stop=False)
                            for b in range(lo, hi):
                                mm(PZ, W(f"W4_b{b}"), h[2][q][:, b, :],
                                   start=False, stop=(b == NBANK - 1))
                            if last:
                                nc.vector.tensor_copy(out49[:, ts(t, BT)], PZ)
                        return go
                    yield l4(0, 5, True, False)
                    yield l4(5, 10, False, False)
                    yield l4(10, NBANK, False, True)
                    return

                gli = li - 1
                gt = {"G1": G1[gli][q], "G2": G2[gli][q],
                      "G1x": G1x[gli][q], "G2x": G2x[gli][q],
                      "G2y": G2y[gli][q]}

                def z0():
                    Ps = [PB[(bank_ctr[0] + i) % 7] for i in range(3)]
                    bank_ctr[0] += 3
                    # T3 first: h0 feeds the next layer's z0 chain
                    for P, name, rows in ((Ps[0], "T3", 64),
                                          (Ps[1], "T1", 128),
                                          (Ps[2], "T2", 128)):
                        if li == 1:
                            mm(P[0:rows, :], W(f"Z1_{name}_k0"), vs(t, 0),
                               start=True, stop=False)
                            mm(P[0:rows, :], W(f"Z1_{name}_k1"), vs(t, 1),
                               start=False, stop=True)
                        else:
                            mm(P[0:rows, :], W64r(f"Z{li}_{name}"),
                               h0[gli - 1][q], start=True, stop=True)
                    nc.scalar.activation(G3[gli][q], Ps[0][0:64, :], Sig)
                    nc.scalar.activation(G1[gli][q], Ps[1], Sig)
                    nc.scalar.activation(G2[gli][q], Ps[2], Sig)
                    TT(out=h0[gli][q], in0=Ps[0][0:64, :], in1=G3[gli][q],
                       op=Mult)
                    # duplicated gate windows (DVE 4x copies) for the
                    # repeated-pattern banks
                    g1, g2 = G1[gli][q], G2[gli][q]
                    cp = nc.vector.tensor_copy
                    cp(G1x[gli][q][0:64, :], g1[0:64, :])
                    cp(G1x[gli][q][64:128, :], g1[0:64, :])
                    cp(G2x[gli][q][0:64, :], g2[0:64, :])
                    cp(G2x[gli][q][64:128, :], g2[0:64, :])
                    # [g6;g4] window straddles a 64-partition boundary:
                    # 64-wide accesses must be 64-aligned, so copy in 32s
                    cp(G2y[gli][q][0:32, :], g2[32:64, :])
                    cp(G2y[gli][q][32:64, :], g2[64:96, :])
                    cp(G2y[gli][q][64:96, :], g2[32:64, :])
                    cp(G2y[gli][q][96:128, :], g2[64:96, :])
                yield z0

                def bank(b):
                    def go():
                        P = PB[bank_ctr[0] % 7]
                        bank_ctr[0] += 1
                        if li == 1:
                            (wu, su), (wl, sl) = L1_PASSES[b]
                            mm(P[0:64, :], W(wu), vs(t, su),
                               start=True, stop=True, tile_position=(0, 0))
                            mm(P[64:128, :], W(wl), vs(t, sl),
                               start=True, stop=True, tile_position=(0, 64))
                        else:
                            mm(P, W(f"L{li}_{L23_MAT[b]}"),
                               h[gli - 1][q][:, b, :], start=True, stop=True)
                        drain(b, P, h[gli][q][:, b, :], gt)
                    return go
                for b in range(NBANK):
                    yield bank(b)

            def emit_phase(streams, head_start=6):
                its = [iter(s) for s in streams]
                for _ in range(head_start):
                    for th in its[:1]:
                        nxt = next(th, None)
                        if nxt:
                            nxt()
                alive = True
                while alive:
                    alive = False
                    for it in its:
                        nxt = next(it, None)
                        if nxt:
                            nxt()
                            alive = True

            # wavefront schedule: (tile, layer) pairs per phase
            SCHED = [[(0, 1)], [(0, 2), (1, 1)], [(0, 3), (1, 2)],
                     [(0, 4), (1, 3)], [(1, 4), (2, 1)], [(2, 2), (3, 1)],
                     [(2, 3), (3, 2)], [(2, 4), (3, 3)], [(3, 4)]]
            for pi, phase in enumerate(SCHED):
                if pi == 2:
                    nc.sync.dma_start(out=vb[0], in_=vt_d[2])
                if pi == 3:
                    nc.sync.dma_start(out=vb[1], in_=vt_d[3])
                emit_phase([layer_stream(t, l) for t, l in phase])
            nc.sync.dma_start(out=out_d, in_=out49)

    _split_excess_waits(nc)
    return nc


def _get_nc(col):
    if "nc" not in _BUILD:
        _BUILD["nc"] = _build_program(col)
    return _BUILD["nc"]


def kernel(v_raw, w1, w2, w3, w4):
    wt, col = _pack_weights(np.asarray(w1, np.float32),
                            np.asarray(w2, np.float32),
                            np.asarray(w3, np.float32),
                            np.asarray(w4, np.float32))
    nc = _get_nc(col)
    vt = _pack_v(np.asarray(v_raw, np.float32))   # [B//BT, 128, NSLOT*BT]
    in_maps = []
    for c in range(NCORES):
        vc = np.ascontiguousarray(vt[c * NT:(c + 1) * NT])
        in_maps.append({"vt": vc, "wt": wt})
    res = bass_utils.run_bass_kernel_spmd(nc, in_maps,
                                          core_ids=list(range(NCORES)))
    global LAST_RESULT
    LAST_RESULT = res
    full = np.empty((B_FULL, D_OUT), np.float32)
    for c in range(NCORES):
        full[c * BC:(c + 1) * BC, :] = res.results[c]["out"].T
    return full.reshape(B_FULL, D_OUT, 1)


# revision 33
# speedup vs baseline: 4.5383x; 1.0136x over previous
"""Trainium2 Bass kernel for nn_EquivariantDecoder.

Data-parallel over 8 NeuronCores (batch sharded, 2048 rows/core).

Fully unrolled program (no For_i loops: each back-edge costs an
all-engine barrier ~2us).  Per 512-row tile:

  - host pre-transposes v into 30 tight matmul-ready [128, BT] slots
    (bf16) per tile; one contiguous DMA per tile (triple-buffered);
  - layer outputs live in 8 PSUM banks: 3 z0 chunks (T1=[g2;g1],
    T2=[g5;g6;g4;g3], T3=[scalars]), 4 rotating hidden banks, 1 L4;
  - gates: Act engine sigmoids z0 psum -> bf16 SBUF tiles G1/G2/G3;
    silu(s) computed as s * sigmoid(s) (keeps Act on one act table);
  - hidden banks [128, BT] are drained psum->SBUF with the gate
    multiply fused (TensorTensor), spread across DVE (1x from psum),
    Pool (0.42 eff), and Act-copy + DVE 2x-bf16 assists;
  - per-irrep weights are packed block-diagonally so every PE pass is
    [<=128K, <=128P] x BT; weight loads are free; 83 passes/tile.
"""

import numpy as np
import ml_dtypes
from contextlib import ExitStack

import concourse.bass as bass
import concourse.mybir as mybir
import concourse.tile as tile
from concourse.bass import ds, ts
from concourse import bass_utils

BF16 = mybir.dt.bfloat16
FP32 = mybir.dt.float32
bf = ml_dtypes.bfloat16

# ---------------- problem constants (hardcoded) ----------------
B_FULL = 16384
NCORES = 8
BC = B_FULL // NCORES          # 2048 rows per core
BT = 512                       # b-tile
NT = BC // BT                  # 4

IN_IRREPS = [(256, 0), (128, 1), (128, 2), (64, 3), (64, 4), (64, 5), (64, 6)]
HID_IRREPS = [(64, 0), (64, 1), (64, 2), (32, 3), (32, 4), (32, 5), (32, 6)]
D_OUT = 49
NSLOT = 30

IN_OFF = {}
_o = 0
for _mul, _l in IN_IRREPS:
    IN_OFF[_l] = _o
    _o += _mul * (2 * _l + 1)

HID_MUL = {l: m for m, l in HID_IRREPS}
IN_MUL = {l: m for m, l in IN_IRREPS}

# v slot map (30 slots of [128, BT]):
#  0,1: l0 channels 0:128 / 128:256
#  2..6: l2 m=0..4 (128 ch)
#  7..9: l1 m=0..2 (128 ch)
#  10..20: [l6_m(64); l5_m(64)] m=0..10
#  21: [l6_11; l4_7]   22: [l6_12; l4_8]
#  23..29: [l4_m(64); l3_m(64)] m=0..6
_BP = []
for _m in range(7):
    _BP += [("p", (6, _m), (5, _m)), ("p", (4, _m), (3, _m))]
_BP += [("p", (6, 7), (5, 7)), ("p", (6, 8), (5, 8)),
        ("p", (6, 9), (5, 9)), ("p", (6, 10), (5, 10)),
        ("p", (6, 11), (4, 7)), ("p", (6, 12), (4, 8))]
V_SLOTS = ([("l0k", 0, None), ("l0k", 1, None)]
           + [("m", 2, m) for m in range(5)]      # l2
           + [("m", 1, m) for m in range(3)]      # l1
           + _BP)

# hidden banks (14): row layout = 2 or 4 (l, m, size) groups
BANK_GROUPS = (
    [[(2, m, 64), (1, m, 64)] for m in range(3)]
    + [[(2, 3, 64), (2, 4, 64)]]
    + [[(5, m, 32), (6, m, 32), (4, m, 32), (3, m, 32)] for m in range(7)]
    + [[(5, 7, 32), (6, 7, 32), (5, 8, 32), (6, 8, 32)],
       [(5, 9, 32), (6, 9, 32), (5, 10, 32), (6, 10, 32)],
       [(6, 11, 32), (4, 7, 32), (6, 12, 32), (4, 8, 32)]]
)
NBANK = 14

# layer-1 passes per bank: (weight name, v slot) for upper/lower half
L1_PASSES = (
    [[("W1_l2", 2 + m), ("W1_l1", 7 + m)] for m in range(3)]
    + [[("W1_l2", 5), ("W1_l2", 6)]]
    + [[("W56", 10 + 2 * m), ("W43", 11 + 2 * m)] for m in range(7)]
    + [[("W56", 24), ("W56", 25)],
       [("W56", 26), ("W56", 27)],
       [("W64", 28), ("W64", 29)]]
)

# layer-2/3 block-diag weight name per bank
L23_MAT = (["A21"] * 3 + ["A22"] + ["B5643"] * 7 + ["B5656"] * 2 + ["B6464"])

# gate windows: ("G1"|"G2", lo, hi, split?)  G1=[g2;g1] G2=[g5;g6;g4;g3]
BANK_GATE = ([("G1", 0, 128, False)] * 3 + [("G1", 0, 64, True)]
             + [("G2", 0, 128, False)] * 7 + [("G2", 0, 64, True)] * 2
             + [("G2", 32, 96, True)])

# drain jobs per bank: list of (kind, half, gate, lo, hi)
#  kind: 'd' DVE TT, 'p' Pool TT, 'a' Act copy + DVE 2x TT,
#        'a2' Act copy + two half 2x TTs, 'dh'/'ph' half TT
# kinds: 'd' DVE TT from psum; 'a' Act copy + DVE 2x TT;
#        'ap' Act copy + Pool TT (GPSIMD cannot read PSUM)
BANK_JOBS = (
    [[("ap", None, "G1", 0, 128)], [("ap", None, "G1", 0, 128)],
     [("d", None, "G1", 0, 128)], [("a", None, "G1x", 0, 128)]]
    + [[("d", None, "G2", 0, 128)], [("d", None, "G2", 0, 128)]]   # 4,5
    + [[("ap", None, "G2", 0, 128)], [("d", None, "G2", 0, 128)]]  # 6,7
    + [[("d", None, "G2", 0, 128)], [("d", None, "G2", 0, 128)],
       [("d", None, "G2", 0, 128)]]                                # 8,9,10
    + [[("ap", None, "G2x", 0, 128)],                              # bank 11
       [("ap", None, "G2x", 0, 128)],                              # bank 12
       [("a", None, "G2y", 0, 128)]]                               # bank 13
)


# gate column windows in the reference's 256-wide gate block
GCOL = {1: (0, 64), 2: (64, 128), 3: (128, 160), 4: (160, 192),
        5: (192, 224), 6: (224, 256)}

_BUILD = {}


def _split_blocks(wflat, in_irr, out_irr):
    mul_in = {l: m for m, l in in_irr}
    blocks = []
    off = 0
    for mo, l in out_irr:
        mi = mul_in[l]
        w = wflat[off:off + mi * mo].reshape(mi, mo) / np.sqrt(mi)
        off += mi * mo
        blocks.append((l, w))
    assert off == wflat.size
    return blocks


def _pack_weights(w1, w2, w3, w4):
    """Pack all weights into one [128, NW] bf16 matrix.
    Returns (wt, col: name -> (col offset, width))."""
    pre_irr = [(64, 0), (256, 0)] + [(m, l) for m, l in HID_IRREPS if l > 0]

    segs = []

    def add(name, arr):
        a = np.zeros((128, arr.shape[1]), np.float32)
        a[:arr.shape[0]] = arr
        segs.append((name, a))

    def z0_chunks(pfx, wflat, in_irr):
        b = _split_blocks(wflat, in_irr, pre_irr)
        ws, wg = b[0][1], b[1][1]
        t1 = np.concatenate([wg[:, GCOL[2][0]:GCOL[2][1]],
                             wg[:, GCOL[1][0]:GCOL[1][1]]], axis=1)
        t2 = np.concatenate([wg[:, GCOL[5][0]:GCOL[5][1]],
                             wg[:, GCOL[6][0]:GCOL[6][1]],
                             wg[:, GCOL[4][0]:GCOL[4][1]],
                             wg[:, GCOL[3][0]:GCOL[3][1]]], axis=1)
        t3 = ws
        K = t1.shape[0]
        if K == 256:
            add(pfx + "T1_k0", t1[0:128]); add(pfx + "T1_k1", t1[128:256])
            add(pfx + "T2_k0", t2[0:128]); add(pfx + "T2_k1", t2[128:256])
            add(pfx + "T3_k0", t3[0:128]); add(pfx + "T3_k1", t3[128:256])
        else:
            add(pfx + "T1", t1); add(pfx + "T2", t2); add(pfx + "T3", t3)
        return {l: w for l, w in b[2:]}

    # ---- layer 1 ----
    wl1 = z0_chunks("Z1_", w1, IN_IRREPS)
    add("W1_l2", wl1[2])                      # [128, 64]
    add("W1_l1", wl1[1])
    w56 = np.zeros((128, 64), np.float32)     # K=[l6;l5] -> P=[l5out;l6out]
    w56[64:128, 0:32] = wl1[5]
    w56[0:64, 32:64] = wl1[6]
    add("W56", w56)
    w43 = np.zeros((128, 64), np.float32)     # K=[l4;l3] -> P=[l4out;l3out]
    w43[0:64, 0:32] = wl1[4]
    w43[64:128, 32:64] = wl1[3]
    add("W43", w43)
    w64 = np.zeros((128, 64), np.float32)     # K=[l6;l4] -> P=[l6out;l4out]
    w64[0:64, 0:32] = wl1[6]
    w64[64:128, 32:64] = wl1[4]
    add("W64", w64)

    # ---- layers 2, 3 ----
    for li, wflat in ((2, w2), (3, w3)):
        wl = z0_chunks(f"Z{li}_", wflat, HID_IRREPS)
        mats = {}
        for name in set(L23_MAT):
            mats[name] = np.zeros((128, 128), np.float32)
        for b in range(NBANK):
            mat = mats[L23_MAT[b]]
            r = 0
            for (l, m, sz) in BANK_GROUPS[b]:
                mat[r:r + sz, r:r + sz] = wl[l]
                r += sz
        for name in ("A21", "A22", "B5643", "B5656", "B6464"):
            add(f"L{li}_{name}", mats[name])

    # ---- layer 4 ----
    b4 = _split_blocks(w4, HID_IRREPS, [(1, l) for l in range(7)])
    w4l = {l: w[:, 0] for l, w in b4}
    w40 = np.zeros((64, D_OUT), np.float32)
    w40[:, 0] = w4l[0]
    add("W4_h0", w40)
    for b in range(NBANK):
        m4 = np.zeros((128, D_OUT), np.float32)
        r = 0
        for (l, m, sz) in BANK_GROUPS[b]:
            m4[r:r + sz, l * l + m] = w4l[l]
            r += sz
        add(f"W4_b{b}", m4)

    col = {}
    off = 0
    for name, a in segs:
        col[name] = (off, a.shape[1])
        off += a.shape[1]
    wt = np.concatenate([a for _, a in segs], axis=1).astype(bf)
    return wt, col


def _pack_v(v_raw):
    """[B, 3840] fp32 -> [B // BT, 128, NSLOT * BT] bf16 (tile-major)."""
    B = v_raw.shape[0]
    vt = np.zeros((128, NSLOT, B), np.float32)

    def blk(l):
        mul = IN_MUL[l]
        d = 2 * l + 1
        return v_raw[:, IN_OFF[l]:IN_OFF[l] + mul * d].reshape(B, mul, d)

    blks = {l: blk(l) for l in range(7)}
    for s, spec in enumerate(V_SLOTS):
        kind = spec[0]
        if kind == "l0k":
            k = spec[1]
            vt[:, s, :] = v_raw[:, 128 * k:128 * (k + 1)].T
        elif kind == "m":
            l, m = spec[1], spec[2]
            vt[:, s, :] = blks[l][:, :, m].T
        else:
            (lu, mu), (ll_, ml) = spec[1], spec[2]
            vt[0:64, s, :] = blks[lu][:, :, mu].T
            vt[64:128, s, :] = blks[ll_][:, :, ml].T
    # -> [ntile, 128, NSLOT, BT] contiguous per tile
    nt = B // BT
    vt = vt.reshape(128, NSLOT, nt, BT).transpose(2, 0, 1, 3)
    return np.ascontiguousarray(vt.reshape(nt, 128, NSLOT * BT).astype(bf))


def _split_excess_waits(nc, max_waits=1):
    """Walrus accepts only one sem-wait on some ops; hoist excess waits
    onto same-engine NoOps inserted before."""
    for f in nc.m.functions:
        for bb in f.blocks:
            newlist = []
            changed = False
            for ins in bb.instructions:
                si = ins.sync_info
                waits = list(si.on_wait) if (si and si.on_wait) else []
                if len(waits) > max_waits:
                    extras, keep = waits[:-max_waits], waits[-max_waits:]
                    for k in range(0, len(extras), max_waits):
                        nop = mybir.InstNoOp(
                            name=f"{ins.name}_waitnop{k}", ins=[], outs=[],
                            engine=ins.engine)
                        nop.sync_info = mybir.SyncInfo(
                            on_wait=extras[k:k + max_waits], on_update=[])
                        nc.register_instruction(nop)
                        newlist.append(nop)
                    ins.sync_info = mybir.SyncInfo(
                        on_wait=keep,
                        on_update=list(si.on_update) if si.on_update else [])
                    changed = True
                newlist.append(ins)
            if changed:
                bb.instructions[:] = newlist
    return nc


def _build_program(col):
    nc = bass.Bass("TRN2", target_bir_lowering=False, debug=False)
    NW = max(o + w for o, w in col.values())
    vt_d = nc.dram_tensor("vt", [NT, 128, NSLOT * BT], BF16,
                          kind="ExternalInput").ap()
    wt_d = nc.dram_tensor("wt", [128, NW], BF16, kind="ExternalInput").ap()
    out_d = nc.dram_tensor("out", [D_OUT, BC], FP32, kind="ExternalOutput").ap()

    Sig = mybir.ActivationFunctionType.Sigmoid
    Copy = mybir.ActivationFunctionType.Copy
    Mult = mybir.AluOpType.mult

    with tile.TileContext(nc) as tc:
        with ExitStack() as ctx:
            pool = ctx.enter_context(tc.tile_pool(name="p", bufs=1))
            pp = ctx.enter_context(tc.tile_pool(name="ps", bufs=1, space="PSUM"))

            wt = pool.tile([128, NW], BF16, tag="wt")
            vb = [pool.tile([128, NSLOT * BT], BF16, tag=f"vb{i}", name=f"vb{i}")
                  for i in range(2)]
            # h[li][parity]: gated hidden after layer li+1
            h = [[pool.tile([128, NBANK, BT], BF16, tag=f"h{i}_{q}",
                            name=f"h{i}_{q}") for q in range(2)]
                 for i in range(3)]
            h0 = [[pool.tile([64, BT], BF16, tag=f"h0{i}_{q}",
                             name=f"h0{i}_{q}") for q in range(2)]
                  for i in range(3)]
            G1 = [[pool.tile([128, BT], BF16, tag=f"G1_{i}_{q}",
                             name=f"G1_{i}_{q}") for q in range(2)]
                  for i in range(3)]
            G2 = [[pool.tile([128, BT], BF16, tag=f"G2_{i}_{q}",
                             name=f"G2_{i}_{q}") for q in range(2)]
                  for i in range(3)]
            G3 = [[pool.tile([64, BT], BF16, tag=f"G3_{i}_{q}",
                             name=f"G3_{i}_{q}") for q in range(2)]
                  for i in range(3)]
            G1x = [[pool.tile([128, BT], BF16, tag=f"G1x_{i}_{q}",
                              name=f"G1x_{i}_{q}") for q in range(2)]
                   for i in range(3)]
            G2x = [[pool.tile([128, BT], BF16, tag=f"G2x_{i}_{q}",
                              name=f"G2x_{i}_{q}") for q in range(2)]
                   for i in range(3)]
            G2y = [[pool.tile([128, BT], BF16, tag=f"G2y_{i}_{q}",
                              name=f"G2y_{i}_{q}") for q in range(2)]
                   for i in range(3)]
            SG = [pool.tile([128, BT], BF16, tag=f"SG{i}", name=f"SG{i}")
                  for i in range(4)]
            out49 = pool.tile([D_OUT, BC], FP32, tag="out49")

            PZ = pp.tile([D_OUT, BT], FP32, tag="PZ")
            PB = [pp.tile([128, BT], FP32, tag=f"PB{i}", name=f"PB{i}")
                  for i in range(7)]

            def W(name):
                c, w = col[name]
                return wt[:, c:c + w]

            def W64r(name, rows=64):
                c, w = col[name]
                return wt[0:rows, c:c + w]

            mm = nc.tensor.matmul
            TT = nc.vector.tensor_tensor
            PTT = nc.gpsimd.tensor_tensor

            c1 = col["W64"][0] + col["W64"][1]   # end of layer-1 weights
            nc.sync.dma_start(out=wt[:, 0:c1], in_=wt_d[:, 0:c1])
            nc.sync.dma_start(out=vb[0][:, 0:2 * BT], in_=vt_d[0][:, 0:2 * BT])
            nc.sync.dma_start(out=vb[0][:, 2 * BT:10 * BT],
                              in_=vt_d[0][:, 2 * BT:10 * BT])
            for a, b in ((10, 18), (18, 26), (26, 30)):
                nc.sync.dma_start(out=vb[0][:, a * BT:b * BT],
                                  in_=vt_d[0][:, a * BT:b * BT])
            nc.sync.dma_start(out=wt[:, c1:], in_=wt_d[:, c1:])
            nc.sync.dma_start(out=vb[1][:, 0:2 * BT], in_=vt_d[1][:, 0:2 * BT])
            nc.sync.dma_start(out=vb[1][:, 2 * BT:10 * BT],
                              in_=vt_d[1][:, 2 * BT:10 * BT])
            for a, b in ((10, 18), (18, 26), (26, 30)):
                nc.sync.dma_start(out=vb[1][:, a * BT:b * BT],
                                  in_=vt_d[1][:, a * BT:b * BT])

            # PE p-state warmup: ramp to max clock while DMAs stream
            dmyw = pool.tile([128, D_OUT], BF16, tag="dmyw")
            dmyx = pool.tile([128, BT], BF16, tag="dmyx")
            nc.vector.memset(dmyw, 0)
            nc.vector.memset(dmyx, 0)
            for i in range(8):
                mm(PZ, dmyw, dmyx, start=(i == 0), stop=(i == 7))

            def vs(t, s):
                return vb[t % 2][:, ds(s * BT, BT)]

            bank_ctr = [0]
            sg_ctr = [0]

            def drain(b, P, hdst, gt):
                for kind, hf, gname, lo, hi in BANK_JOBS[b]:
                    g = gt[gname]
                    if kind in ("a", "ap"):
                        sg = SG[sg_ctr[0] % 4]
                        sg_ctr[0] += 1
                        nc.scalar.activation(sg, P, Copy)
                        e = TT if kind == "a" else PTT
                        e(out=hdst, in0=sg, in1=g[lo:hi, :], op=Mult)
                    else:  # 'd'
                        TT(out=hdst, in0=P, in1=g[lo:hi, :], op=Mult)

            def layer_stream(t, li):
                """Generator of emission thunks for (tile t, layer li in 1..4)."""
                q = t % 2
                if li == 4:
                    def l4(lo, hi, first, last):
                        def go():
                            if first:
                                mm(PZ, W64r("W4_h0"), h0[2][q],
                                   start=True, stop=False)
                            for b in range(lo, hi):
                                mm(PZ, W(f"W4_b{b}"), h[2][q][:, b, :],
                                   start=False, stop=(b == NBANK - 1))
                            if last:
                                nc.scalar.activation(out49[:, ts(t, BT)],
                                                     PZ, Copy)
                        return go
                    yield l4(0, 5, True, False)
                    yield l4(5, 10, False, False)
                    yield l4(10, NBANK, False, True)
                    return

                gli = li - 1
                gt = {"G1": G1[gli][q], "G2": G2[gli][q],
                      "G1x": G1x[gli][q], "G2x": G2x[gli][q],
                      "G2y": G2y[gli][q]}

                def z0():
                    Ps = [PB[(bank_ctr[0] + i) % 7] for i in range(3)]
                    bank_ctr[0] += 3
                    # T3 first: h0 feeds the next layer's z0 chain
                    for P, name, rows in ((Ps[0], "T3", 64),
                                          (Ps[1], "T1", 128),
                                          (Ps[2], "T2", 128)):
                        if li == 1:
                            mm(P[0:rows, :], W(f"Z1_{name}_k0"), vs(t, 0),
                               start=True, stop=False)
                            mm(P[0:rows, :], W(f"Z1_{name}_k1"), vs(t, 1),
                               start=False, stop=True)
                        else:
                            mm(P[0:rows, :], W64r(f"Z{li}_{name}"),
                               h0[gli - 1][q], start=True, stop=True)
                    nc.scalar.activation(G3[gli][q], Ps[0][0:64, :], Sig)
                    nc.scalar.activation(G1[gli][q], Ps[1], Sig)
                    nc.scalar.activation(G2[gli][q], Ps[2], Sig)
                    TT(out=h0[gli][q], in0=Ps[0][0:64, :], in1=G3[gli][q],
                       op=Mult)
                    # duplicated gate windows (DVE 4x copies) for the
                    # repeated-pattern banks
                    g1, g2 = G1[gli][q], G2[gli][q]
                    cp = nc.vector.tensor_copy
                    cp(G1x[gli][q][0:64, :], g1[0:64, :])
                    cp(G1x[gli][q][64:128, :], g1[0:64, :])
                    cp(G2x[gli][q][0:64, :], g2[0:64, :])
                    cp(G2x[gli][q][64:128, :], g2[0:64, :])
                    # [g6;g4] window straddles a 64-partition boundary:
                    # 64-wide accesses must be 64-aligned, so copy in 32s
                    cp(G2y[gli][q][0:32, :], g2[32:64, :])
                    cp(G2y[gli][q][32:64, :], g2[64:96, :])
                    cp(G2y[gli][q][64:96, :], g2[32:64, :])
                    cp(G2y[gli][q][96:128, :], g2[64:96, :])
                yield z0

                def bank(b):
                    def go():
                        P = PB[bank_ctr[0] % 7]
                        bank_ctr[0] += 1
                        if li == 1:
                            (wu, su), (wl, sl) = L1_PASSES[b]
                            mm(P[0:64, :], W(wu), vs(t, su),
                               start=True, stop=True, tile_position=(0, 0))
                            mm(P[64:128, :], W(wl), vs(t, sl),
                               start=True, stop=True, tile_position=(0, 64))
                        else:
                            mm(P, W(f"L{li}_{L23_MAT[b]}"),
                               h[gli - 1][q][:, b, :], start=True, stop=True)
                        drain(b, P, h[gli][q][:, b, :], gt)
                    return go
                for b in range(NBANK):
                    yield bank(b)

            def emit_phase(streams, head_start=6):
                its = [iter(s) for s in streams]
                for _ in range(head_start):
                    for th in its[:1]:
                        nxt = next(th, None)
                        if nxt:
                            nxt()
                alive = True
                while alive:
                    alive = False
                    for it in its:
                        nxt = next(it, None)
                        if nxt:
                            nxt()
                            alive = True

            # wavefront schedule: (tile, layer) pairs per phase
            SCHED = [[(0, 1)], [(0, 2), (1, 1)], [(0, 3), (1, 2)],
                     [(0, 4), (1, 3)], [(1, 4), (2, 1)], [(2, 2), (3, 1)],
                     [(2, 3), (3, 2)], [(2, 4), (3, 3)], [(3, 4)]]
            for pi, phase in enumerate(SCHED):
                if pi == 2:
                    nc.sync.dma_start(out=vb[0], in_=vt_d[2])
                if pi == 3:
                    nc.sync.dma_start(out=vb[1], in_=vt_d[3])
                hs = 2 if phase[0][1] == 4 else 6
                emit_phase([layer_stream(t, l) for t, l in phase],
                           head_start=hs)
            nc.sync.dma_start(out=out_d, in_=out49)

    _split_excess_waits(nc)
    return nc


def _get_nc(col):
    if "nc" not in _BUILD:
        _BUILD["nc"] = _build_program(col)
    return _BUILD["nc"]


def kernel(v_raw, w1, w2, w3, w4):
    wt, col = _pack_weights(np.asarray(w1, np.float32),
                            np.asarray(w2, np.float32),
                            np.asarray(w3, np.float32),
                            np.asarray(w4, np.float32))
    nc = _get_nc(col)
    vt = _pack_v(np.asarray(v_raw, np.float32))   # [B//BT, 128, NSLOT*BT]
    in_maps = []
    for c in range(NCORES):
        vc = np.ascontiguousarray(vt[c * NT:(c + 1) * NT])
        in_maps.append({"vt": vc, "wt": wt})
    res = bass_utils.run_bass_kernel_spmd(nc, in_maps,
                                          core_ids=list(range(NCORES)))
    global LAST_RESULT
    LAST_RESULT = res
    full = np.empty((B_FULL, D_OUT), np.float32)
    for c in range(NCORES):
        full[c * BC:(c + 1) * BC, :] = res.results[c]["out"].T
    return full.reshape(B_FULL, D_OUT, 1)
